# revision 40
# baseline (speedup 1.0000x reference)
"""Trainium2 Bass kernel for HandDecoder-style GNN message passing.

Math (per batch element b):
  f = relu(MLP3([feature, coords]))                              # [N, C1=32]
  t1[i,j,h] = relu(a[j,h] + kb1[h] - a[i,h]),  a = coords @ kw1  # [N,N,8]
  t2[i,j,k] = relu(sum_h t1[i,j,h] kw2[h,k] + kb2[k])           # [N,N,16]
  g[j,k,d]  = sum_c f[j,c] kw3[k, c*16+d]                       # [N,16,16]
  out[i,d]  = relu(sum_{j,k} t2[i,j,k] g[j,k,d] + sum_c F[c] kb3[c*16+d])
  (F[c] = sum_j f[j,c])

Layouts (per core, BPC=4 batches):
  t1 tiles [p=(jl16,h8)=128, i=128] per chunk jc in 0..8 (16 j's each):
    PE mm1: lhsT=sel128 [8,128] (delta(h(p),h')), rhs=-a [8, i] (x4 rep)
    a2b[j,h] added as per-partition BIAS during the relu PSUM->SBUF copy
    (a2b columns [128, 32] built by one kron matmul: L = I16 (x) [kw1;kb1]).
  t2 tiles [q=(jl8,k16)=128, i=128] per c2 in 0..16 (8 j's each):
    lhsT = block-diag kw2 (lo/hi halves), bias kb2 + relu on copy.
  g bounced through DRAM early (j-partitions -> (jl,k)-partitions).
  final: out[i,(b,d)] accumulates 16 chunk matmuls + rank-1 bias2.
Data-parallel over batch: 4 per core, 8 cores. bf16 matmuls (fp32r for the
decode MLP), fp32 accumulation in PSUM.
"""

import sys
import numpy as np

for _p in ("/opt/trn_rl_repo",):
    if _p not in sys.path:
        sys.path.insert(0, _p)

import concourse.bass as bass
import concourse.bacc as bacc
import concourse.mybir as mybir
import concourse.tile as tile
from concourse.bass_utils import run_bass_kernel_spmd

B, N = 32, 128
C0, C1, C2 = 64, 32, 16
NCORES = 8
BPC = B // NCORES          # batches per core = 4
F32 = mybir.dt.float32
F32R = mybir.dt.float32r
BF16 = mybir.dt.bfloat16
RELU = mybir.ActivationFunctionType.Relu
COPY = mybir.ActivationFunctionType.Copy
ADD = mybir.AluOpType.add
MAX = mybir.AluOpType.max

_CACHED_NC = None

# wb (fp32 [128, 256]) column map
WB_DW1 = 0          # [67, 0:32]
WB_DW2 = 32         # [32, 32:48]
WB_DW3 = 48         # [16, 48:80]
WB_L = 80           # [64, 80:208] kron L
WB_KW1N = 208       # [4, 208:216] = [-kw1; 0]
WB_DB1 = 216
WB_DB2 = 217
WB_DB3 = 218
WB_KB2T = 219       # [128, 219:220] kb2 tiled per (jl8, k)
WB_KB3R = 224       # [0:32, 224:240] kb3 reshaped [32, 16]
WB_COLS = 240

# wh (bf16) column map
WH_LO = 0           # [128, 0:128]
WH_HI = 128         # [128, 128:256]
WH_KW3P = 256       # [32, 256:512] kw3 permuted; kb3r directly after
WH_KB3R = 512       # [32, 512:528]
WH_SEL = 528        # [8, 528:656] sel128: delta(h(p), h')
WH_ONES = 656       # [1, 656:784] ones row (rank-1 bias matmul lhsT)
WH_EYE = 784        # [32, 784:816] eye32
WH_DW1 = 816        # [67, 816:848]
WH_DW2 = 848        # [32, 848:864]
WH_DW3 = 864        # [16, 864:896]
WH_KW1N = 896       # [4, 896:904] = [-kw1; 0]
WH_COLS = 904


def build_nc(stage=5):
    import os
    stage = int(os.environ.get("KSTAGE", stage))
    nc = bacc.Bacc("TRN2", target_bir_lowering=False, debug=False,
                   num_devices=NCORES)

    xT_d = nc.dram_tensor("xT", [67, BPC * N], BF16, kind="ExternalInput").ap()
    c4T_d = nc.dram_tensor("c4T", [4, BPC * N], BF16, kind="ExternalInput").ap()
    c4R_d = nc.dram_tensor("c4R", [64, BPC * 8], F32, kind="ExternalInput").ap()
    wb_d = nc.dram_tensor("wb", [128, WB_COLS], F32, kind="ExternalInput").ap()
    wh_d = nc.dram_tensor("wh", [128, WH_COLS], BF16, kind="ExternalInput").ap()
    out_d = nc.dram_tensor("out", [N, BPC, C2], F32, kind="ExternalOutput").ap()
    g_dram = nc.dram_tensor("gscr", [BPC * 128 * 256], BF16).ap()

    with tile.TileContext(nc) as tc:
        with (
            tc.tile_pool(name="const", bufs=1) as cpool,
            tc.tile_pool(name="work", bufs=1) as wpool,
            tc.tile_pool(name="ps_m", bufs=2,
                         space=bass.MemorySpace.PSUM) as pmisc,
            tc.tile_pool(name="ps_t1", bufs=3,
                         space=bass.MemorySpace.PSUM) as pt1,
            tc.tile_pool(name="ps_t2", bufs=3,
                         space=bass.MemorySpace.PSUM) as pt2,
        ):
            # ---- input DMAs. v1 cost: issuing engine busy for the whole
            # transfer; transfer ns = dst free-bytes/partition * 0.386 ----
            xT_s = cpool.tile([67, BPC * N], BF16, tag="xT")
            c4T_s = cpool.tile([4, BPC * N], BF16, tag="c4T")
            c4R_s = cpool.tile([64, BPC * 8], F32, tag="c4R")
            wb_s = cpool.tile([128, WB_COLS], F32, tag="wb")
            wh_s = cpool.tile([128, WH_COLS], BF16, tag="wh")
            nc.scalar.dma_start(c4R_s[:], c4R_d)
            nc.sync.dma_start(wb_s[:], wb_d)
            nc.gpsimd.dma_start(c4T_s[:], c4T_d)
            nc.sync.dma_start(xT_s[:], xT_d)
            nc.gpsimd.dma_start(wh_s[:], wh_d)

            dw1 = wh_s[0:67, WH_DW1:WH_DW1 + 32]
            dw2 = wh_s[0:32, WH_DW2:WH_DW2 + 16]
            dw3 = wh_s[0:16, WH_DW3:WH_DW3 + 32]
            kronL = wb_s[0:64, WB_L:WB_L + 128]
            kw1n4 = wh_s[0:4, WH_KW1N:WH_KW1N + 8]
            db1 = wb_s[0:32, WB_DB1:WB_DB1 + 1]
            db2 = wb_s[0:16, WB_DB2:WB_DB2 + 1]
            db3 = wb_s[0:32, WB_DB3:WB_DB3 + 1]
            kb2t = wb_s[0:128, WB_KB2T:WB_KB2T + 1]
            kb3r = wb_s[0:32, WB_KB3R:WB_KB3R + 16]
            lo = wh_s[0:128, WH_LO:WH_LO + 128]
            hi = wh_s[0:128, WH_HI:WH_HI + 128]
            kw3p = wh_s[0:32, WH_KW3P:WH_KW3P + 256]
            sel128 = wh_s[0:8, WH_SEL:WH_SEL + 128]
            ones_r = wh_s[0:1, WH_ONES:WH_ONES + 128]
            eye32 = wh_s[0:32, WH_EYE:WH_EYE + 32]

            # SBUF working tiles
            g_rm = wpool.tile([128, BPC * 256], BF16, tag="grm")
            g_all = wpool.tile([128, BPC * 256], BF16, tag="gall")
            bias2f = wpool.tile([1, BPC * C2], F32, tag="b2f")
            bias2_sb = wpool.tile([1, BPC * C2], BF16, tag="b2s")
            a2bT_sb = wpool.tile([BPC * 8, 128], BF16, tag="a2bT")
            aneg_sb = wpool.tile([8, BPC * N], BF16, tag="aneg")
            h1 = wpool.tile([32, BPC * N], BF16, tag="h1")
            h2 = wpool.tile([16, BPC * N], BF16, tag="h2")
            fT = wpool.tile([32, BPC * N], BF16, tag="fT")
            t1_sb = [wpool.tile([128, 8 * N], BF16, tag=f"t1b{b}",
                                name=f"t1b{b}") for b in range(BPC)]
            t2_sb = [wpool.tile([128, 16 * N], BF16, tag=f"t2b{b}",
                                name=f"t2b{b}") for b in range(BPC)]
            out_sb = wpool.tile([128, BPC * C2], F32, tag="osb")

            # ---- PE warmup: ramp p-state while DMAs land ----
            warm_sb = wpool.tile([1, 512], BF16, tag="warm")
            nc.vector.memset(warm_sb[:], 0.0)
            for _ in range(2):
                wps = pmisc.tile([1, 512], F32, tag="m")
                nc.tensor.matmul(wps[:], warm_sb[0:1, 0:1], warm_sb[:])

            # ---- aneg = -(coords @ kw1): [8, (b,i)] -> bf16 ----
            aneg_ps = pmisc.tile([8, BPC * N], F32, tag="m")
            nc.tensor.matmul(aneg_ps[:], kw1n4, c4T_s[:])
            for b in range(BPC):
                nc.vector.tensor_copy(aneg_sb[0:8, b * N:(b + 1) * N],
                                      aneg_ps[0:8, b * N:(b + 1) * N])

            # ---- a2bT: one kron matmul -> [32, (jl,h)=128] ----
            a2bT_ps = pmisc.tile([BPC * 8, 128], F32, tag="m")
            nc.tensor.matmul(a2bT_ps[:], c4R_s[:], kronL)
            nc.scalar.activation(a2bT_sb[:], a2bT_ps[:], COPY)

            def t1_mms(b):
                rhs_rep = aneg_sb[0:8, b * N:(b + 1) * N] \
                    .unsqueeze(1).broadcast_to([8, 4, N])
                tiles = []
                for half in range(2):
                    t1p = pt1.tile([128, 512], F32, tag="t1ps",
                                   name=f"t1p{b}_{half}")
                    nc.tensor.matmul(t1p[:], sel128, rhs_rep,
                                     start=True, stop=False)
                    c0 = b * 8 + half * 4
                    rhs1 = eye32[0:32, c0:c0 + 4] \
                        .unsqueeze(2).broadcast_to([32, 4, N])
                    nc.tensor.matmul(t1p[:], a2bT_sb[:], rhs1,
                                     start=False, stop=True)
                    tiles.append(t1p)
                return tiles

            t1_engs = [[nc.vector, nc.scalar], [nc.scalar, nc.vector],
                       [nc.vector, nc.scalar], [nc.scalar, nc.vector]]

            def t1_copies(b, tiles):
                for half in range(2):
                    dst = t1_sb[b][:, half * 512:(half + 1) * 512]
                    eng = t1_engs[b][half]
                    if eng is nc.scalar:
                        nc.scalar.activation(dst, tiles[half][:], RELU)
                    else:
                        eng.tensor_scalar(dst, tiles[half][:], 0.0, None, MAX)

            # ---- decode MLP (fp32r), t1 matmuls slotted into the gaps ----
            d1_ps = pmisc.tile([32, BPC * N], F32, tag="m")
            nc.tensor.matmul(d1_ps[:], dw1, xT_s[:])
            t1t0 = t1_mms(0)
            nc.vector.tensor_scalar(h1[:], d1_ps[:], db1, 0.0, ADD, MAX)
            t1_copies(0, t1t0)
            d2_ps = pmisc.tile([16, BPC * N], F32, tag="m")
            nc.tensor.matmul(d2_ps[:], dw2, h1[:])
            t1t1 = t1_mms(1)
            nc.vector.tensor_scalar(h2[:], d2_ps[:], db2, 0.0, ADD, MAX)
            t1_copies(1, t1t1)
            t1t2 = t1_mms(2)
            t1_copies(2, t1t2)
            d3_ps = pmisc.tile([32, BPC * N], F32, tag="m")
            nc.tensor.matmul(d3_ps[:], dw3, h2[:])
            t1t3 = t1_mms(3)
            t1_copies(3, t1t3)
            nc.scalar.activation(fT[:], d3_ps[:], RELU, bias=db3)

            # ---- g (+fused bias2 cols): per b [128, 272] ----
            g_engs = [nc.scalar, nc.vector, nc.scalar, nc.vector]
            F_sb = wpool.tile([32, BPC], F32, tag="F")
            for b in range(BPC):
                gps = pmisc.tile([128, 256], F32, tag="m")
                nc.tensor.matmul(gps[:], fT[0:32, b * N:(b + 1) * N], kw3p)
                if g_engs[b] is nc.scalar:
                    nc.scalar.activation(g_rm[:, b * 256:(b + 1) * 256],
                                         gps[:], COPY)
                else:
                    g_engs[b].tensor_copy(g_rm[:, b * 256:(b + 1) * 256],
                                          gps[:])
                nc.vector.tensor_reduce(F_sb[:, b:b + 1],
                                        fT[0:32, b * N:(b + 1) * N],
                                        mybir.AxisListType.X, ADD)
            bias2_ps = pmisc.tile([1, BPC * C2], F32, tag="m")
            for b in range(BPC):
                nc.tensor.matmul(bias2_ps[0:1, b * C2:(b + 1) * C2],
                                 F_sb[0:32, b:b + 1], kb3r)
            nc.scalar.activation(bias2_sb[:], bias2_ps[:], COPY)
            # bounce A: g_rm -> dram (dst flat: ~500ns each)
            for bp in range(2):
                srcA = g_rm[:, bp * 512:(bp + 1) * 512].rearrange(
                    "p (b c) -> p b c", b=2)
                dstA = g_dram[bp * 65536:(bp + 1) * 65536].rearrange(
                    "(b j c) -> j b c", j=128, b=2)
                nc.sync.dma_start(dstA, srcA)
            # bounce B: dram -> g_all[(jl8,k), (b, c2, d)]
            for b in range(BPC):
                dstB = g_all[:, b * 256:(b + 1) * 256].rearrange(
                    "p (c d) -> p c d", d=16)
                srcB = g_dram[b * 32768:(b + 1) * 32768].rearrange(
                    "(c jk d) -> jk c d", jk=128, d=16)
                eng = nc.sync if b % 2 == 0 else nc.gpsimd
                eng.dma_start(dstB, srcB)

            # ---- t2 + finals (finals delayed one batch) ----
            out_ps = pmisc.tile([128, BPC * C2], F32, tag="m")
            t2_engs = [[nc.scalar, nc.vector, nc.vector, nc.scalar],
                       [nc.vector, nc.scalar, nc.scalar, nc.vector],
                       [nc.scalar, nc.vector, nc.vector, nc.scalar],
                       [nc.vector, nc.scalar, nc.scalar, nc.vector]]

            def finals(fb):
                for c2 in range(16):
                    nc.tensor.matmul(
                        out_ps[:, fb * C2:(fb + 1) * C2],
                        t2_sb[fb][:, c2 * N:(c2 + 1) * N],
                        g_all[:, fb * 256 + c2 * 16: fb * 256 + (c2 + 1) * 16],
                        start=(c2 == 0), stop=False)
                nc.tensor.matmul(out_ps[:, fb * C2:(fb + 1) * C2],
                                 ones_r, bias2_sb[0:1, fb * C2:(fb + 1) * C2],
                                 start=False, stop=True)
                if fb % 2 == 1:
                    sl = slice((fb - 1) * C2, (fb + 1) * C2)
                    nc.scalar.activation(out_sb[:, sl], out_ps[:, sl], RELU)
                    eng = nc.sync if fb == 1 else nc.gpsimd
                    eng.dma_start(out_d[:, fb - 1:fb + 1, :],
                                  out_sb[:, sl].rearrange(
                                      "p (b d) -> p b d", b=2))

            for b in range(BPC):
                for tp in range(4):
                    ps = pt2.tile([128, 512], F32, tag="t2ps",
                                  name=f"t2p{b}_{tp}")
                    for q in range(4):
                        c2 = tp * 4 + q
                        jc, half2 = c2 // 2, c2 % 2
                        lhsT = lo if half2 == 0 else hi
                        nc.tensor.matmul(
                            ps[:, q * N:(q + 1) * N], lhsT,
                            t1_sb[b][:, jc * N:(jc + 1) * N])
                    dst = t2_sb[b][:, tp * 512:(tp + 1) * 512]
                    eng = t2_engs[b][tp]
                    if eng is nc.scalar:
                        nc.scalar.activation(dst, ps[:], RELU, bias=kb2t)
                    else:
                        eng.tensor_scalar(dst, ps[:], kb2t, 0.0, ADD, MAX)
                if b > 0:
                    finals(b - 1)
            finals(BPC - 1)

    nc.compile()
    return nc


def _host_inputs(feature, coordinates_v, dw1, db1, dw2, db2, dw3, db3,
                 kw1, kb1, kw2, kb2, kw3, kb3):
    """Per-core input maps. Pure layout transforms, no FLOPs."""
    f32, bf16 = np.float32, None
    import ml_dtypes
    bf16 = ml_dtypes.bfloat16

    wb = np.zeros((128, WB_COLS), f32)
    wb[0:67, WB_DW1:WB_DW1 + 32] = dw1
    wb[0:32, WB_DW2:WB_DW2 + 16] = dw2
    wb[0:16, WB_DW3:WB_DW3 + 32] = dw3
    # kron L[jl*4+x, jl2*8+h] = (jl==jl2) * kw1b4[x, h]
    kw1b4 = np.zeros((4, 8), f32)
    kw1b4[0:3] = kw1
    kw1b4[3] = kb1
    L = np.zeros((64, 128), f32)
    for jl in range(16):
        L[jl * 4:(jl + 1) * 4, jl * 8:(jl + 1) * 8] = kw1b4
    wb[0:64, WB_L:WB_L + 128] = L
    wb[0:3, WB_KW1N:WB_KW1N + 8] = -kw1
    wb[0:32, WB_DB1] = db1
    wb[0:16, WB_DB2] = db2
    wb[0:32, WB_DB3] = db3
    wb[:, WB_KB2T] = np.tile(kb2, 8)
    wb[0:32, WB_KB3R:WB_KB3R + 16] = kb3.reshape(32, 16)


    wh = np.zeros((128, WH_COLS), f32)
    # lo/hi: lo[jl*8+h, jl2*16+k] = (jl==jl2)*kw2[h,k]  (jl2 in 0..8)
    for jl2 in range(8):
        wh[jl2 * 8:(jl2 + 1) * 8, WH_LO + jl2 * 16:WH_LO + (jl2 + 1) * 16] = kw2
        wh[64 + jl2 * 8:64 + (jl2 + 1) * 8,
           WH_HI + jl2 * 16:WH_HI + (jl2 + 1) * 16] = kw2
    wh[0:32, WH_KW3P:WH_KW3P + 256] = \
        kw3.reshape(16, 32, 16).transpose(1, 0, 2).reshape(32, 256)
    wh[0:32, WH_KB3R:WH_KB3R + 16] = kb3.reshape(32, 16)
    # sel128[h, jl*8+h2] = (h==h2)
    cols = np.arange(128)
    wh[0:8, WH_SEL:WH_SEL + 128] = \
        (cols[None, :] % 8 == np.arange(8)[:, None]).astype(f32)
    wh[0:1, WH_ONES:WH_ONES + 128] = 1.0
    wh[0:32, WH_EYE:WH_EYE + 32] = np.eye(32, dtype=f32)
    wh[0:67, WH_DW1:WH_DW1 + 32] = dw1
    wh[0:32, WH_DW2:WH_DW2 + 16] = dw2
    wh[0:16, WH_DW3:WH_DW3 + 32] = dw3
    wh[0:3, WH_KW1N:WH_KW1N + 8] = -kw1
    wh = wh.astype(bf16)

    in_maps = []
    for c in range(NCORES):
        fe = feature[c * BPC:(c + 1) * BPC]          # [4, 64]
        co = coordinates_v[c * BPC:(c + 1) * BPC]    # [4, 128, 3]
        xT = np.empty((67, BPC * N), f32)
        c4T = np.empty((4, BPC * N), f32)
        for b in range(BPC):
            xT[0:64, b * N:(b + 1) * N] = fe[b][:, None]
            xT[64:67, b * N:(b + 1) * N] = co[b].T
            c4T[0:3, b * N:(b + 1) * N] = co[b].T
        c4T[3, :] = 1.0
        # c4R[jl*4+x, b*8+jc] = coords4[b, jc*16+jl, x]
        co4 = np.concatenate([co, np.ones((BPC, N, 1), f32)], axis=2)
        c4R = co4.reshape(BPC, 8, 16, 4).transpose(2, 3, 0, 1) \
            .reshape(64, BPC * 8)
        in_maps.append({"xT": np.ascontiguousarray(xT).astype(bf16),
                        "c4T": np.ascontiguousarray(c4T).astype(bf16),
                        "c4R": np.ascontiguousarray(c4R),
                        "wb": wb, "wh": wh})
    return in_maps


def kernel(**inputs):
    global _CACHED_NC
    if _CACHED_NC is None:
        _CACHED_NC = build_nc()
    nc = _CACHED_NC
    in_maps = _host_inputs(
        np.asarray(inputs["feature"]), np.asarray(inputs["coordinates_v"]),
        np.asarray(inputs["dw1"]), np.asarray(inputs["db1"]),
        np.asarray(inputs["dw2"]), np.asarray(inputs["db2"]),
        np.asarray(inputs["dw3"]), np.asarray(inputs["db3"]),
        np.asarray(inputs["kw1"]), np.asarray(inputs["kb1"]),
        np.asarray(inputs["kw2"]), np.asarray(inputs["kb2"]),
        np.asarray(inputs["kw3"]), np.asarray(inputs["kb3"]))
    res = run_bass_kernel_spmd(nc, in_maps, list(range(NCORES)))
    out = np.empty((B, N, C2), np.float32)
    for c in range(NCORES):
        # per-core out is [N(i), BPC(b), C2(d)]
        out[c * BPC:(c + 1) * BPC] = res.results[c]["out"].transpose(1, 0, 2)
    return out


# revision 42
# speedup vs baseline: 1.0151x; 1.0151x over previous
"""Trainium2 Bass kernel for HandDecoder-style GNN message passing.

Math (per batch element b):
  f = relu(MLP3([feature, coords]))                              # [N, C1=32]
  t1[i,j,h] = relu(a[j,h] + kb1[h] - a[i,h]),  a = coords @ kw1  # [N,N,8]
  t2[i,j,k] = relu(sum_h t1[i,j,h] kw2[h,k] + kb2[k])           # [N,N,16]
  g[j,k,d]  = sum_c f[j,c] kw3[k, c*16+d]                       # [N,16,16]
  out[i,d]  = relu(sum_{j,k} t2[i,j,k] g[j,k,d] + sum_c F[c] kb3[c*16+d])
  (F[c] = sum_j f[j,c])

Layouts (per core, BPC=4 batches):
  t1 tiles [p=(jl16,h8)=128, i=128] per chunk jc in 0..8 (16 j's each):
    PE mm1: lhsT=sel128 [8,128] (delta(h(p),h')), rhs=-a [8, i] (x4 rep)
    a2b[j,h] added as per-partition BIAS during the relu PSUM->SBUF copy
    (a2b columns [128, 32] built by one kron matmul: L = I16 (x) [kw1;kb1]).
  t2 tiles [q=(jl8,k16)=128, i=128] per c2 in 0..16 (8 j's each):
    lhsT = block-diag kw2 (lo/hi halves), bias kb2 + relu on copy.
  g bounced through DRAM early (j-partitions -> (jl,k)-partitions).
  final: out[i,(b,d)] accumulates 16 chunk matmuls + rank-1 bias2.
Data-parallel over batch: 4 per core, 8 cores. bf16 matmuls (fp32r for the
decode MLP), fp32 accumulation in PSUM.
"""

import sys
import numpy as np

for _p in ("/opt/trn_rl_repo",):
    if _p not in sys.path:
        sys.path.insert(0, _p)

import concourse.bass as bass
import concourse.bacc as bacc
import concourse.mybir as mybir
import concourse.tile as tile
from concourse.bass_utils import run_bass_kernel_spmd

B, N = 32, 128
C0, C1, C2 = 64, 32, 16
NCORES = 8
BPC = B // NCORES          # batches per core = 4
F32 = mybir.dt.float32
F32R = mybir.dt.float32r
BF16 = mybir.dt.bfloat16
RELU = mybir.ActivationFunctionType.Relu
COPY = mybir.ActivationFunctionType.Copy
ADD = mybir.AluOpType.add
MAX = mybir.AluOpType.max

_CACHED_NC = None

# wb (fp32 [128, 256]) column map
WB_DW1 = 0          # [67, 0:32]
WB_DW2 = 32         # [32, 32:48]
WB_DW3 = 48         # [16, 48:80]
WB_L = 80           # [64, 80:208] kron L
WB_KW1N = 208       # [4, 208:216] = [-kw1; 0]
WB_DB1 = 216
WB_DB2 = 217
WB_DB3 = 218
WB_KB2T = 219       # [128, 219:220] kb2 tiled per (jl8, k)
WB_KB3R = 224       # [0:32, 224:240] kb3 reshaped [32, 16]
WB_COLS = 240

# wh (bf16) column map
WH_LO = 0           # [128, 0:128]
WH_HI = 128         # [128, 128:256]
WH_KW3P = 256       # [32, 256:512] kw3 permuted; kb3r directly after
WH_KB3R = 512       # [32, 512:528]
WH_SEL = 528        # [8, 528:656] sel128: delta(h(p), h')
WH_ONES = 656       # [1, 656:784] ones row (rank-1 bias matmul lhsT)
WH_EYE = 784        # [32, 784:816] eye32
WH_DW1 = 816        # [67, 816:848]
WH_DW2 = 848        # [32, 848:864]
WH_DW3 = 864        # [16, 864:896]
WH_KW1N = 896       # [4, 896:904] = [-kw1; 0]
WH_COLS = 904


def build_nc(stage=5):
    import os
    stage = int(os.environ.get("KSTAGE", stage))
    nc = bacc.Bacc("TRN2", target_bir_lowering=False, debug=False,
                   num_devices=NCORES)

    xT_d = nc.dram_tensor("xT", [67, BPC * N], BF16, kind="ExternalInput").ap()
    c4T_d = nc.dram_tensor("c4T", [4, BPC * N], BF16, kind="ExternalInput").ap()
    c4R_d = nc.dram_tensor("c4R", [64, BPC * 8], F32, kind="ExternalInput").ap()
    wb_d = nc.dram_tensor("wb", [128, WB_COLS], F32, kind="ExternalInput").ap()
    wh_d = nc.dram_tensor("wh", [128, WH_COLS], BF16, kind="ExternalInput").ap()
    out_d = nc.dram_tensor("out", [N, BPC, C2], F32, kind="ExternalOutput").ap()
    g_dram = nc.dram_tensor("gscr", [BPC * 128 * 256], BF16).ap()

    with tile.TileContext(nc) as tc:
        with (
            tc.tile_pool(name="const", bufs=1) as cpool,
            tc.tile_pool(name="work", bufs=1) as wpool,
            tc.tile_pool(name="ps_m", bufs=2,
                         space=bass.MemorySpace.PSUM) as pmisc,
            tc.tile_pool(name="ps_t1", bufs=3,
                         space=bass.MemorySpace.PSUM) as pt1,
            tc.tile_pool(name="ps_t2", bufs=3,
                         space=bass.MemorySpace.PSUM) as pt2,
        ):
            # ---- input DMAs. v1 cost: issuing engine busy for the whole
            # transfer; transfer ns = dst free-bytes/partition * 0.386 ----
            xT_s = cpool.tile([67, BPC * N], BF16, tag="xT")
            c4T_s = cpool.tile([4, BPC * N], BF16, tag="c4T")
            c4R_s = cpool.tile([64, BPC * 8], F32, tag="c4R")
            wb_s = cpool.tile([128, WB_COLS], F32, tag="wb")
            wh_s = cpool.tile([128, WH_COLS], BF16, tag="wh")
            nc.scalar.dma_start(c4R_s[:], c4R_d)
            nc.sync.dma_start(wb_s[:], wb_d)
            nc.gpsimd.dma_start(c4T_s[:], c4T_d)
            nc.sync.dma_start(xT_s[:], xT_d)
            nc.gpsimd.dma_start(wh_s[:], wh_d)

            dw1 = wh_s[0:67, WH_DW1:WH_DW1 + 32]
            dw2 = wh_s[0:32, WH_DW2:WH_DW2 + 16]
            dw3 = wh_s[0:16, WH_DW3:WH_DW3 + 32]
            kronL = wb_s[0:64, WB_L:WB_L + 128]
            kw1n4 = wh_s[0:4, WH_KW1N:WH_KW1N + 8]
            db1 = wb_s[0:32, WB_DB1:WB_DB1 + 1]
            db2 = wb_s[0:16, WB_DB2:WB_DB2 + 1]
            db3 = wb_s[0:32, WB_DB3:WB_DB3 + 1]
            kb2t = wb_s[0:128, WB_KB2T:WB_KB2T + 1]
            kb3r = wb_s[0:32, WB_KB3R:WB_KB3R + 16]
            lo = wh_s[0:128, WH_LO:WH_LO + 128]
            hi = wh_s[0:128, WH_HI:WH_HI + 128]
            kw3p = wh_s[0:32, WH_KW3P:WH_KW3P + 256]
            sel128 = wh_s[0:8, WH_SEL:WH_SEL + 128]
            ones_r = wh_s[0:1, WH_ONES:WH_ONES + 128]
            eye32 = wh_s[0:32, WH_EYE:WH_EYE + 32]

            # SBUF working tiles
            g_rm = wpool.tile([128, BPC * 256], BF16, tag="grm")
            g_all = wpool.tile([128, BPC * 256], BF16, tag="gall")
            bias2f = wpool.tile([1, BPC * C2], F32, tag="b2f")
            bias2_sb = wpool.tile([1, BPC * C2], BF16, tag="b2s")
            a2bT_sb = wpool.tile([BPC * 8, 128], BF16, tag="a2bT")
            aneg_sb = wpool.tile([8, BPC * N], BF16, tag="aneg")
            h1 = wpool.tile([32, BPC * N], BF16, tag="h1")
            h2 = wpool.tile([16, BPC * N], BF16, tag="h2")
            fT = wpool.tile([32, BPC * N], BF16, tag="fT")
            t1_sb = [wpool.tile([128, 8 * N], BF16, tag=f"t1b{b}",
                                name=f"t1b{b}") for b in range(BPC)]
            t2_sb = [wpool.tile([128, 16 * N], BF16, tag=f"t2b{b}",
                                name=f"t2b{b}") for b in range(BPC)]
            out_sb = wpool.tile([128, BPC * C2], F32, tag="osb")

            # ---- PE warmup: ramp p-state while DMAs land ----
            warm_sb = wpool.tile([1, 512], BF16, tag="warm")
            nc.vector.memset(warm_sb[:], 0.0)
            for _ in range(2):
                wps = pmisc.tile([1, 512], F32, tag="m")
                nc.tensor.matmul(wps[:], warm_sb[0:1, 0:1], warm_sb[:])

            # ---- aneg = -(coords @ kw1): [8, (b,i)] -> bf16 ----
            aneg_ps = pmisc.tile([8, BPC * N], F32, tag="m")
            nc.tensor.matmul(aneg_ps[:], kw1n4, c4T_s[:])
            for b in range(BPC):
                nc.vector.tensor_copy(aneg_sb[0:8, b * N:(b + 1) * N],
                                      aneg_ps[0:8, b * N:(b + 1) * N])

            # ---- a2bT: one kron matmul -> [32, (jl,h)=128] ----
            a2bT_ps = pmisc.tile([BPC * 8, 128], F32, tag="m")
            nc.tensor.matmul(a2bT_ps[:], c4R_s[:], kronL)
            nc.scalar.activation(a2bT_sb[:], a2bT_ps[:], COPY)

            def t1_mms(b):
                rhs_rep = aneg_sb[0:8, b * N:(b + 1) * N] \
                    .unsqueeze(1).broadcast_to([8, 4, N])
                tiles = []
                for half in range(2):
                    t1p = pt1.tile([128, 512], F32, tag="t1ps",
                                   name=f"t1p{b}_{half}")
                    nc.tensor.matmul(t1p[:], sel128, rhs_rep,
                                     start=True, stop=False)
                    c0 = b * 8 + half * 4
                    rhs1 = eye32[0:32, c0:c0 + 4] \
                        .unsqueeze(2).broadcast_to([32, 4, N])
                    nc.tensor.matmul(t1p[:], a2bT_sb[:], rhs1,
                                     start=False, stop=True)
                    tiles.append(t1p)
                return tiles

            t1_engs = [[nc.scalar, nc.vector], [nc.vector, nc.scalar],
                       [nc.scalar, nc.vector], [nc.vector, nc.scalar]]

            def t1_copies(b, tiles):
                for half in range(2):
                    dst = t1_sb[b][:, half * 512:(half + 1) * 512]
                    eng = t1_engs[b][half]
                    if eng is nc.scalar:
                        nc.scalar.activation(dst, tiles[half][:], RELU)
                    else:
                        eng.tensor_scalar(dst, tiles[half][:], 0.0, None, MAX)

            # ---- decode MLP (fp32r), t1 matmuls slotted into the gaps ----
            d1_ps = pmisc.tile([32, BPC * N], F32, tag="m")
            nc.tensor.matmul(d1_ps[:], dw1, xT_s[:])
            t1t0 = t1_mms(0)
            nc.vector.tensor_scalar(h1[:], d1_ps[:], db1, 0.0, ADD, MAX)
            t1_copies(0, t1t0)
            d2_ps = pmisc.tile([16, BPC * N], F32, tag="m")
            nc.tensor.matmul(d2_ps[:], dw2, h1[:])
            t1t1 = t1_mms(1)
            nc.vector.tensor_scalar(h2[:], d2_ps[:], db2, 0.0, ADD, MAX)
            t1_copies(1, t1t1)
            t1t2 = t1_mms(2)
            t1_copies(2, t1t2)
            d3_ps = pmisc.tile([32, BPC * N], F32, tag="m")
            nc.tensor.matmul(d3_ps[:], dw3, h2[:])
            t1t3 = t1_mms(3)
            t1_copies(3, t1t3)
            nc.scalar.activation(fT[:], d3_ps[:], RELU, bias=db3)

            # ---- g (+fused bias2 cols): per b [128, 272] ----
            g_engs = [nc.scalar, nc.vector, nc.scalar, nc.vector]
            F_sb = wpool.tile([32, BPC], F32, tag="F")
            for b in range(BPC):
                gps = pmisc.tile([128, 256], F32, tag="m")
                nc.tensor.matmul(gps[:], fT[0:32, b * N:(b + 1) * N], kw3p)
                if g_engs[b] is nc.scalar:
                    nc.scalar.activation(g_rm[:, b * 256:(b + 1) * 256],
                                         gps[:], COPY)
                else:
                    g_engs[b].tensor_copy(g_rm[:, b * 256:(b + 1) * 256],
                                          gps[:])
                nc.vector.tensor_reduce(F_sb[:, b:b + 1],
                                        fT[0:32, b * N:(b + 1) * N],
                                        mybir.AxisListType.X, ADD)
            bias2_ps = pmisc.tile([1, BPC * C2], F32, tag="m")
            for b in range(BPC):
                nc.tensor.matmul(bias2_ps[0:1, b * C2:(b + 1) * C2],
                                 F_sb[0:32, b:b + 1], kb3r)
            nc.scalar.activation(bias2_sb[:], bias2_ps[:], COPY)
            # bounce A: g_rm -> dram (dst flat: ~500ns each)
            for bp in range(2):
                srcA = g_rm[:, bp * 512:(bp + 1) * 512].rearrange(
                    "p (b c) -> p b c", b=2)
                dstA = g_dram[bp * 65536:(bp + 1) * 65536].rearrange(
                    "(b j c) -> j b c", j=128, b=2)
                nc.sync.dma_start(dstA, srcA)
            # bounce B: dram -> g_all[(jl8,k), (b, c2, d)]
            for b in range(BPC):
                dstB = g_all[:, b * 256:(b + 1) * 256].rearrange(
                    "p (c d) -> p c d", d=16)
                srcB = g_dram[b * 32768:(b + 1) * 32768].rearrange(
                    "(c jk d) -> jk c d", jk=128, d=16)
                eng = nc.sync if b % 2 == 0 else nc.gpsimd
                eng.dma_start(dstB, srcB)

            # ---- t2 + finals (finals delayed one batch) ----
            out_ps = pmisc.tile([128, BPC * C2], F32, tag="m")
            t2_engs = [[nc.vector, nc.scalar, nc.scalar, nc.vector],
                       [nc.scalar, nc.vector, nc.vector, nc.scalar],
                       [nc.vector, nc.scalar, nc.scalar, nc.vector],
                       [nc.scalar, nc.vector, nc.vector, nc.scalar]]

            def finals(fb):
                for c2 in range(16):
                    nc.tensor.matmul(
                        out_ps[:, fb * C2:(fb + 1) * C2],
                        t2_sb[fb][:, c2 * N:(c2 + 1) * N],
                        g_all[:, fb * 256 + c2 * 16: fb * 256 + (c2 + 1) * 16],
                        start=(c2 == 0), stop=False)
                nc.tensor.matmul(out_ps[:, fb * C2:(fb + 1) * C2],
                                 ones_r, bias2_sb[0:1, fb * C2:(fb + 1) * C2],
                                 start=False, stop=True)
                if fb % 2 == 1:
                    sl = slice((fb - 1) * C2, (fb + 1) * C2)
                    nc.scalar.activation(out_sb[:, sl], out_ps[:, sl], RELU)
                    eng = nc.sync if fb == 1 else nc.gpsimd
                    eng.dma_start(out_d[:, fb - 1:fb + 1, :],
                                  out_sb[:, sl].rearrange(
                                      "p (b d) -> p b d", b=2))

            for b in range(BPC):
                for tp in range(4):
                    ps = pt2.tile([128, 512], F32, tag="t2ps",
                                  name=f"t2p{b}_{tp}")
                    for q in range(4):
                        c2 = tp * 4 + q
                        jc, half2 = c2 // 2, c2 % 2
                        lhsT = lo if half2 == 0 else hi
                        nc.tensor.matmul(
                            ps[:, q * N:(q + 1) * N], lhsT,
                            t1_sb[b][:, jc * N:(jc + 1) * N])
                    dst = t2_sb[b][:, tp * 512:(tp + 1) * 512]
                    eng = t2_engs[b][tp]
                    if eng is nc.scalar:
                        nc.scalar.activation(dst, ps[:], RELU, bias=kb2t)
                    else:
                        eng.tensor_scalar(dst, ps[:], kb2t, 0.0, ADD, MAX)
                if b > 0:
                    finals(b - 1)
            finals(BPC - 1)

    nc.compile()
    return nc


def _host_inputs(feature, coordinates_v, dw1, db1, dw2, db2, dw3, db3,
                 kw1, kb1, kw2, kb2, kw3, kb3):
    """Per-core input maps. Pure layout transforms, no FLOPs."""
    f32, bf16 = np.float32, None
    import ml_dtypes
    bf16 = ml_dtypes.bfloat16

    wb = np.zeros((128, WB_COLS), f32)
    wb[0:67, WB_DW1:WB_DW1 + 32] = dw1
    wb[0:32, WB_DW2:WB_DW2 + 16] = dw2
    wb[0:16, WB_DW3:WB_DW3 + 32] = dw3
    # kron L[jl*4+x, jl2*8+h] = (jl==jl2) * kw1b4[x, h]
    kw1b4 = np.zeros((4, 8), f32)
    kw1b4[0:3] = kw1
    kw1b4[3] = kb1
    L = np.zeros((64, 128), f32)
    for jl in range(16):
        L[jl * 4:(jl + 1) * 4, jl * 8:(jl + 1) * 8] = kw1b4
    wb[0:64, WB_L:WB_L + 128] = L
    wb[0:3, WB_KW1N:WB_KW1N + 8] = -kw1
    wb[0:32, WB_DB1] = db1
    wb[0:16, WB_DB2] = db2
    wb[0:32, WB_DB3] = db3
    wb[:, WB_KB2T] = np.tile(kb2, 8)
    wb[0:32, WB_KB3R:WB_KB3R + 16] = kb3.reshape(32, 16)


    wh = np.zeros((128, WH_COLS), f32)
    # lo/hi: lo[jl*8+h, jl2*16+k] = (jl==jl2)*kw2[h,k]  (jl2 in 0..8)
    for jl2 in range(8):
        wh[jl2 * 8:(jl2 + 1) * 8, WH_LO + jl2 * 16:WH_LO + (jl2 + 1) * 16] = kw2
        wh[64 + jl2 * 8:64 + (jl2 + 1) * 8,
           WH_HI + jl2 * 16:WH_HI + (jl2 + 1) * 16] = kw2
    wh[0:32, WH_KW3P:WH_KW3P + 256] = \
        kw3.reshape(16, 32, 16).transpose(1, 0, 2).reshape(32, 256)
    wh[0:32, WH_KB3R:WH_KB3R + 16] = kb3.reshape(32, 16)
    # sel128[h, jl*8+h2] = (h==h2)
    cols = np.arange(128)
    wh[0:8, WH_SEL:WH_SEL + 128] = \
        (cols[None, :] % 8 == np.arange(8)[:, None]).astype(f32)
    wh[0:1, WH_ONES:WH_ONES + 128] = 1.0
    wh[0:32, WH_EYE:WH_EYE + 32] = np.eye(32, dtype=f32)
    wh[0:67, WH_DW1:WH_DW1 + 32] = dw1
    wh[0:32, WH_DW2:WH_DW2 + 16] = dw2
    wh[0:16, WH_DW3:WH_DW3 + 32] = dw3
    wh[0:3, WH_KW1N:WH_KW1N + 8] = -kw1
    wh = wh.astype(bf16)

    in_maps = []
    for c in range(NCORES):
        fe = feature[c * BPC:(c + 1) * BPC]          # [4, 64]
        co = coordinates_v[c * BPC:(c + 1) * BPC]    # [4, 128, 3]
        xT = np.empty((67, BPC * N), f32)
        c4T = np.empty((4, BPC * N), f32)
        for b in range(BPC):
            xT[0:64, b * N:(b + 1) * N] = fe[b][:, None]
            xT[64:67, b * N:(b + 1) * N] = co[b].T
            c4T[0:3, b * N:(b + 1) * N] = co[b].T
        c4T[3, :] = 1.0
        # c4R[jl*4+x, b*8+jc] = coords4[b, jc*16+jl, x]
        co4 = np.concatenate([co, np.ones((BPC, N, 1), f32)], axis=2)
        c4R = co4.reshape(BPC, 8, 16, 4).transpose(2, 3, 0, 1) \
            .reshape(64, BPC * 8)
        in_maps.append({"xT": np.ascontiguousarray(xT).astype(bf16),
                        "c4T": np.ascontiguousarray(c4T).astype(bf16),
                        "c4R": np.ascontiguousarray(c4R),
                        "wb": wb, "wh": wh})
    return in_maps


def kernel(**inputs):
    global _CACHED_NC
    if _CACHED_NC is None:
        _CACHED_NC = build_nc()
    nc = _CACHED_NC
    in_maps = _host_inputs(
        np.asarray(inputs["feature"]), np.asarray(inputs["coordinates_v"]),
        np.asarray(inputs["dw1"]), np.asarray(inputs["db1"]),
        np.asarray(inputs["dw2"]), np.asarray(inputs["db2"]),
        np.asarray(inputs["dw3"]), np.asarray(inputs["db3"]),
        np.asarray(inputs["kw1"]), np.asarray(inputs["kb1"]),
        np.asarray(inputs["kw2"]), np.asarray(inputs["kb2"]),
        np.asarray(inputs["kw3"]), np.asarray(inputs["kb3"]))
    res = run_bass_kernel_spmd(nc, in_maps, list(range(NCORES)))
    out = np.empty((B, N, C2), np.float32)
    for c in range(NCORES):
        # per-core out is [N(i), BPC(b), C2(d)]
        out[c * BPC:(c + 1) * BPC] = res.results[c]["out"].transpose(1, 0, 2)
    return out


# revision 47
# speedup vs baseline: 1.0181x; 1.0029x over previous
"""Trainium2 Bass kernel for HandDecoder-style GNN message passing.

Math (per batch element b):
  f = relu(MLP3([feature, coords]))                              # [N, C1=32]
  t1[i,j,h] = relu(a[j,h] + kb1[h] - a[i,h]),  a = coords @ kw1  # [N,N,8]
  t2[i,j,k] = relu(sum_h t1[i,j,h] kw2[h,k] + kb2[k])           # [N,N,16]
  g[j,k,d]  = sum_c f[j,c] kw3[k, c*16+d]                       # [N,16,16]
  out[i,d]  = relu(sum_{j,k} t2[i,j,k] g[j,k,d] + sum_c F[c] kb3[c*16+d])
  (F[c] = sum_j f[j,c])

Layouts (per core, BPC=4 batches):
  t1 tiles [p=(jl16,h8)=128, i=128] per chunk jc in 0..8 (16 j's each):
    PE mm1: lhsT=sel128 [8,128] (delta(h(p),h')), rhs=-a [8, i] (x4 rep)
    a2b[j,h] added as per-partition BIAS during the relu PSUM->SBUF copy
    (a2b columns [128, 32] built by one kron matmul: L = I16 (x) [kw1;kb1]).
  t2 tiles [q=(jl8,k16)=128, i=128] per c2 in 0..16 (8 j's each):
    lhsT = block-diag kw2 (lo/hi halves), bias kb2 + relu on copy.
  g bounced through DRAM early (j-partitions -> (jl,k)-partitions).
  final: out[i,(b,d)] accumulates 16 chunk matmuls + rank-1 bias2.
Data-parallel over batch: 4 per core, 8 cores. bf16 matmuls (fp32r for the
decode MLP), fp32 accumulation in PSUM.
"""

import sys
import numpy as np

for _p in ("/opt/trn_rl_repo",):
    if _p not in sys.path:
        sys.path.insert(0, _p)

import concourse.bass as bass
import concourse.bacc as bacc
import concourse.mybir as mybir
import concourse.tile as tile
from concourse.bass_utils import run_bass_kernel_spmd

B, N = 32, 128
C0, C1, C2 = 64, 32, 16
NCORES = 8
BPC = B // NCORES          # batches per core = 4
F32 = mybir.dt.float32
F32R = mybir.dt.float32r
BF16 = mybir.dt.bfloat16
RELU = mybir.ActivationFunctionType.Relu
COPY = mybir.ActivationFunctionType.Copy
ADD = mybir.AluOpType.add
MAX = mybir.AluOpType.max

_CACHED_NC = None

# wb (fp32 [128, 256]) column map
WB_DW1 = 0          # [67, 0:32]
WB_DW2 = 32         # [32, 32:48]
WB_DW3 = 48         # [16, 48:80]
WB_L = 80           # [64, 80:208] kron L
WB_KW1N = 208       # [4, 208:216] = [-kw1; 0]
WB_DB1 = 216
WB_DB2 = 217
WB_DB3 = 218
WB_KB2T = 219       # [128, 219:220] kb2 tiled per (jl8, k)
WB_KB3R = 224       # [0:32, 224:240] kb3 reshaped [32, 16]
WB_COLS = 240

# wh (bf16) column map
WH_LO = 0           # [128, 0:128]
WH_HI = 128         # [128, 128:256]
WH_KW3P = 256       # [32, 256:512] kw3 permuted; kb3r directly after
WH_KB3R = 512       # [32, 512:528]
WH_SEL = 528        # [8, 528:656] sel128: delta(h(p), h')
WH_ONES = 656       # [1, 656:784] ones row (rank-1 bias matmul lhsT)
WH_EYE = 784        # [32, 784:816] eye32
WH_DW1 = 816        # [67, 816:848]
WH_DW2 = 848        # [32, 848:864]
WH_DW3 = 864        # [16, 864:896]
WH_KW1N = 896       # [4, 896:904] = [-kw1; 0]
WH_COLS = 904


def build_nc(stage=5):
    import os
    stage = int(os.environ.get("KSTAGE", stage))
    nc = bacc.Bacc("TRN2", target_bir_lowering=False, debug=False,
                   num_devices=NCORES)

    xT_d = nc.dram_tensor("xT", [67, BPC * N], BF16, kind="ExternalInput").ap()
    c4T_d = nc.dram_tensor("c4T", [4, BPC * N], BF16, kind="ExternalInput").ap()
    c4R_d = nc.dram_tensor("c4R", [64, BPC * 8], F32, kind="ExternalInput").ap()
    wb_d = nc.dram_tensor("wb", [128, WB_COLS], F32, kind="ExternalInput").ap()
    wh_d = nc.dram_tensor("wh", [128, WH_COLS], BF16, kind="ExternalInput").ap()
    out_d = nc.dram_tensor("out", [N, BPC, C2], F32, kind="ExternalOutput").ap()
    g_dram = nc.dram_tensor("gscr", [BPC * 128 * 256], BF16).ap()

    with tile.TileContext(nc) as tc:
        with (
            tc.tile_pool(name="const", bufs=1) as cpool,
            tc.tile_pool(name="work", bufs=1) as wpool,
            tc.tile_pool(name="ps_m", bufs=2,
                         space=bass.MemorySpace.PSUM) as pmisc,
            tc.tile_pool(name="ps_t1", bufs=3,
                         space=bass.MemorySpace.PSUM) as pt1,
            tc.tile_pool(name="ps_t2", bufs=3,
                         space=bass.MemorySpace.PSUM) as pt2,
        ):
            # ---- input DMAs. v1 cost: issuing engine busy for the whole
            # transfer; transfer ns = dst free-bytes/partition * 0.386 ----
            xT_s = cpool.tile([67, BPC * N], BF16, tag="xT")
            c4T_s = cpool.tile([4, BPC * N], BF16, tag="c4T")
            c4R_s = cpool.tile([64, BPC * 8], F32, tag="c4R")
            wb_s = cpool.tile([128, WB_COLS], F32, tag="wb")
            wh_s = cpool.tile([128, WH_COLS], BF16, tag="wh")
            nc.scalar.dma_start(c4R_s[:], c4R_d)
            nc.sync.dma_start(wb_s[:], wb_d)
            nc.gpsimd.dma_start(c4T_s[:], c4T_d)
            nc.sync.dma_start(xT_s[:], xT_d)
            nc.gpsimd.dma_start(wh_s[:], wh_d)

            dw1 = wh_s[0:67, WH_DW1:WH_DW1 + 32]
            dw2 = wh_s[0:32, WH_DW2:WH_DW2 + 16]
            dw3 = wh_s[0:16, WH_DW3:WH_DW3 + 32]
            kronL = wb_s[0:64, WB_L:WB_L + 128]
            kw1n4 = wh_s[0:4, WH_KW1N:WH_KW1N + 8]
            db1 = wb_s[0:32, WB_DB1:WB_DB1 + 1]
            db2 = wb_s[0:16, WB_DB2:WB_DB2 + 1]
            db3 = wb_s[0:32, WB_DB3:WB_DB3 + 1]
            kb2t = wb_s[0:128, WB_KB2T:WB_KB2T + 1]
            kb3r = wb_s[0:32, WB_KB3R:WB_KB3R + 16]
            lo = wh_s[0:128, WH_LO:WH_LO + 128]
            hi = wh_s[0:128, WH_HI:WH_HI + 128]
            kw3p = wh_s[0:32, WH_KW3P:WH_KW3P + 256]
            sel128 = wh_s[0:8, WH_SEL:WH_SEL + 128]
            ones_r = wh_s[0:1, WH_ONES:WH_ONES + 128]
            eye32 = wh_s[0:32, WH_EYE:WH_EYE + 32]

            # SBUF working tiles
            g_rm = wpool.tile([128, BPC * 256], BF16, tag="grm")
            g_all = wpool.tile([128, BPC * 256], BF16, tag="gall")
            bias2f = wpool.tile([1, BPC * C2], F32, tag="b2f")
            bias2_sb = wpool.tile([1, BPC * C2], BF16, tag="b2s")
            a2bT_sb = wpool.tile([BPC * 8, 128], BF16, tag="a2bT")
            aneg_sb = wpool.tile([8, BPC * N], BF16, tag="aneg")
            h1 = wpool.tile([32, BPC * N], BF16, tag="h1")
            h2 = wpool.tile([16, BPC * N], BF16, tag="h2")
            fT = wpool.tile([32, BPC * N], BF16, tag="fT")
            t1_sb = [wpool.tile([128, 8 * N], BF16, tag=f"t1b{b}",
                                name=f"t1b{b}") for b in range(BPC)]
            t2_sb = [wpool.tile([128, 16 * N], BF16, tag=f"t2b{b}",
                                name=f"t2b{b}") for b in range(BPC)]
            out_sb = wpool.tile([128, BPC * C2], F32, tag="osb")

            # ---- PE warmup: ramp p-state while DMAs land ----
            warm_sb = wpool.tile([1, 512], BF16, tag="warm")
            nc.vector.memset(warm_sb[:], 0.0)
            for _ in range(2):
                wps = pmisc.tile([1, 512], F32, tag="m")
                nc.tensor.matmul(wps[:], warm_sb[0:1, 0:1], warm_sb[:])

            # ---- aneg = -(coords @ kw1): [8, (b,i)] -> bf16 ----
            aneg_ps = pmisc.tile([8, BPC * N], F32, tag="m")
            nc.tensor.matmul(aneg_ps[:], kw1n4, c4T_s[:])
            for b in range(BPC):
                nc.vector.tensor_copy(aneg_sb[0:8, b * N:(b + 1) * N],
                                      aneg_ps[0:8, b * N:(b + 1) * N])

            # ---- a2bT: one kron matmul -> [32, (jl,h)=128] ----
            a2bT_ps = pmisc.tile([BPC * 8, 128], F32, tag="m")
            nc.tensor.matmul(a2bT_ps[:], c4R_s[:], kronL)
            nc.scalar.activation(a2bT_sb[:], a2bT_ps[:], COPY)

            def t1_mms(b):
                rhs_rep = aneg_sb[0:8, b * N:(b + 1) * N] \
                    .unsqueeze(1).broadcast_to([8, 4, N])
                tiles = []
                for half in range(2):
                    t1p = pt1.tile([128, 512], F32, tag="t1ps",
                                   name=f"t1p{b}_{half}")
                    nc.tensor.matmul(t1p[:], sel128, rhs_rep,
                                     start=True, stop=False)
                    c0 = b * 8 + half * 4
                    rhs1 = eye32[0:32, c0:c0 + 4] \
                        .unsqueeze(2).broadcast_to([32, 4, N])
                    nc.tensor.matmul(t1p[:], a2bT_sb[:], rhs1,
                                     start=False, stop=True)
                    tiles.append(t1p)
                return tiles

            t1_engs = [[nc.scalar, nc.vector], [nc.vector, nc.scalar],
                       [nc.scalar, nc.vector], [nc.vector, nc.scalar]]

            def t1_copies(b, tiles):
                for half in range(2):
                    dst = t1_sb[b][:, half * 512:(half + 1) * 512]
                    eng = t1_engs[b][half]
                    if eng is nc.scalar:
                        nc.scalar.activation(dst, tiles[half][:], RELU)
                    else:
                        eng.tensor_scalar(dst, tiles[half][:], 0.0, None, MAX)

            # ---- decode MLP (fp32r), t1 matmuls slotted into the gaps ----
            d1_ps = pmisc.tile([32, BPC * N], F32, tag="m")
            nc.tensor.matmul(d1_ps[:], dw1, xT_s[:])
            t1t0 = t1_mms(0)
            nc.vector.tensor_scalar(h1[:], d1_ps[:], db1, 0.0, ADD, MAX)
            t1_copies(0, t1t0)
            d2_ps = pmisc.tile([16, BPC * N], F32, tag="m")
            nc.tensor.matmul(d2_ps[:], dw2, h1[:])
            t1t1 = t1_mms(1)
            nc.vector.tensor_scalar(h2[:], d2_ps[:], db2, 0.0, ADD, MAX)
            t1_copies(1, t1t1)
            t1t2 = t1_mms(2)
            t1_copies(2, t1t2)
            d3_ps = pmisc.tile([32, BPC * N], F32, tag="m")
            nc.tensor.matmul(d3_ps[:], dw3, h2[:])
            t1t3 = t1_mms(3)
            t1_copies(3, t1t3)
            nc.scalar.activation(fT[:], d3_ps[:], RELU, bias=db3)

            # ---- g (+fused bias2 cols): per b [128, 272] ----
            g_engs = [nc.scalar, nc.vector, nc.scalar, nc.vector]
            F_sb = wpool.tile([32, BPC], F32, tag="F")
            for b in range(BPC):
                gps = pmisc.tile([128, 256], F32, tag="m")
                nc.tensor.matmul(gps[:], fT[0:32, b * N:(b + 1) * N], kw3p)
                if g_engs[b] is nc.scalar:
                    nc.scalar.activation(g_rm[:, b * 256:(b + 1) * 256],
                                         gps[:], COPY)
                else:
                    g_engs[b].tensor_copy(g_rm[:, b * 256:(b + 1) * 256],
                                          gps[:])
                nc.vector.tensor_reduce(F_sb[:, b:b + 1],
                                        fT[0:32, b * N:(b + 1) * N],
                                        mybir.AxisListType.X, ADD)
            bias2_ps = pmisc.tile([1, BPC * C2], F32, tag="m")
            for b in range(BPC):
                nc.tensor.matmul(bias2_ps[0:1, b * C2:(b + 1) * C2],
                                 F_sb[0:32, b:b + 1], kb3r)
            nc.scalar.activation(bias2_sb[:], bias2_ps[:], COPY)
            # bounce A: g_rm -> dram (dst flat: ~500ns each)
            for bp in range(2):
                srcA = g_rm[:, bp * 512:(bp + 1) * 512].rearrange(
                    "p (b c) -> p b c", b=2)
                dstA = g_dram[bp * 65536:(bp + 1) * 65536].rearrange(
                    "(b j c) -> j b c", j=128, b=2)
                nc.sync.dma_start(dstA, srcA)
            # bounce B: dram -> g_all[(jl8,k), (b, c2, d)]
            for b in range(BPC):
                dstB = g_all[:, b * 256:(b + 1) * 256].rearrange(
                    "p (c d) -> p c d", d=16)
                srcB = g_dram[b * 32768:(b + 1) * 32768].rearrange(
                    "(c jk d) -> jk c d", jk=128, d=16)
                eng = nc.sync if b % 2 == 0 else nc.gpsimd
                eng.dma_start(dstB, srcB)

            # ---- t2 + finals (finals delayed one batch) ----
            out_ps = pmisc.tile([128, BPC * C2], F32, tag="m")
            t2_engs = [[nc.vector, nc.scalar, nc.scalar, nc.vector],
                       [nc.scalar, nc.vector, nc.vector, nc.scalar],
                       [nc.vector, nc.scalar, nc.scalar, nc.vector],
                       [nc.scalar, nc.vector, nc.vector, nc.scalar]]

            def finals(fb):
                for c2 in range(16):
                    nc.tensor.matmul(
                        out_ps[:, fb * C2:(fb + 1) * C2],
                        t2_sb[fb][:, c2 * N:(c2 + 1) * N],
                        g_all[:, fb * 256 + c2 * 16: fb * 256 + (c2 + 1) * 16],
                        start=(c2 == 0), stop=False)
                nc.tensor.matmul(out_ps[:, fb * C2:(fb + 1) * C2],
                                 ones_r, bias2_sb[0:1, fb * C2:(fb + 1) * C2],
                                 start=False, stop=True)
                if fb % 2 == 1:
                    sl = slice((fb - 1) * C2, (fb + 1) * C2)
                    nc.vector.tensor_scalar(out_sb[:, sl], out_ps[:, sl],
                                            0.0, None, MAX)
                    eng = nc.sync if fb == 1 else nc.gpsimd
                    eng.dma_start(out_d[:, fb - 1:fb + 1, :],
                                  out_sb[:, sl].rearrange(
                                      "p (b d) -> p b d", b=2))

            for b in range(BPC):
                for tp in range(4):
                    ps = pt2.tile([128, 512], F32, tag="t2ps",
                                  name=f"t2p{b}_{tp}")
                    for q in range(4):
                        c2 = tp * 4 + q
                        jc, half2 = c2 // 2, c2 % 2
                        lhsT = lo if half2 == 0 else hi
                        nc.tensor.matmul(
                            ps[:, q * N:(q + 1) * N], lhsT,
                            t1_sb[b][:, jc * N:(jc + 1) * N])
                    dst = t2_sb[b][:, tp * 512:(tp + 1) * 512]
                    eng = t2_engs[b][tp]
                    if eng is nc.scalar:
                        nc.scalar.activation(dst, ps[:], RELU, bias=kb2t)
                    else:
                        eng.tensor_scalar(dst, ps[:], kb2t, 0.0, ADD, MAX)
                if b > 0:
                    finals(b - 1)
            finals(BPC - 1)

    nc.compile()
    return nc


def _host_inputs(feature, coordinates_v, dw1, db1, dw2, db2, dw3, db3,
                 kw1, kb1, kw2, kb2, kw3, kb3):
    """Per-core input maps. Pure layout transforms, no FLOPs."""
    f32, bf16 = np.float32, None
    import ml_dtypes
    bf16 = ml_dtypes.bfloat16

    wb = np.zeros((128, WB_COLS), f32)
    wb[0:67, WB_DW1:WB_DW1 + 32] = dw1
    wb[0:32, WB_DW2:WB_DW2 + 16] = dw2
    wb[0:16, WB_DW3:WB_DW3 + 32] = dw3
    # kron L[jl*4+x, jl2*8+h] = (jl==jl2) * kw1b4[x, h]
    kw1b4 = np.zeros((4, 8), f32)
    kw1b4[0:3] = kw1
    kw1b4[3] = kb1
    L = np.zeros((64, 128), f32)
    for jl in range(16):
        L[jl * 4:(jl + 1) * 4, jl * 8:(jl + 1) * 8] = kw1b4
    wb[0:64, WB_L:WB_L + 128] = L
    wb[0:3, WB_KW1N:WB_KW1N + 8] = -kw1
    wb[0:32, WB_DB1] = db1
    wb[0:16, WB_DB2] = db2
    wb[0:32, WB_DB3] = db3
    wb[:, WB_KB2T] = np.tile(kb2, 8)
    wb[0:32, WB_KB3R:WB_KB3R + 16] = kb3.reshape(32, 16)


    wh = np.zeros((128, WH_COLS), f32)
    # lo/hi: lo[jl*8+h, jl2*16+k] = (jl==jl2)*kw2[h,k]  (jl2 in 0..8)
    for jl2 in range(8):
        wh[jl2 * 8:(jl2 + 1) * 8, WH_LO + jl2 * 16:WH_LO + (jl2 + 1) * 16] = kw2
        wh[64 + jl2 * 8:64 + (jl2 + 1) * 8,
           WH_HI + jl2 * 16:WH_HI + (jl2 + 1) * 16] = kw2
    wh[0:32, WH_KW3P:WH_KW3P + 256] = \
        kw3.reshape(16, 32, 16).transpose(1, 0, 2).reshape(32, 256)
    wh[0:32, WH_KB3R:WH_KB3R + 16] = kb3.reshape(32, 16)
    # sel128[h, jl*8+h2] = (h==h2)
    cols = np.arange(128)
    wh[0:8, WH_SEL:WH_SEL + 128] = \
        (cols[None, :] % 8 == np.arange(8)[:, None]).astype(f32)
    wh[0:1, WH_ONES:WH_ONES + 128] = 1.0
    wh[0:32, WH_EYE:WH_EYE + 32] = np.eye(32, dtype=f32)
    wh[0:67, WH_DW1:WH_DW1 + 32] = dw1
    wh[0:32, WH_DW2:WH_DW2 + 16] = dw2
    wh[0:16, WH_DW3:WH_DW3 + 32] = dw3
    wh[0:3, WH_KW1N:WH_KW1N + 8] = -kw1
    wh = wh.astype(bf16)

    in_maps = []
    for c in range(NCORES):
        fe = feature[c * BPC:(c + 1) * BPC]          # [4, 64]
        co = coordinates_v[c * BPC:(c + 1) * BPC]    # [4, 128, 3]
        xT = np.empty((67, BPC * N), f32)
        c4T = np.empty((4, BPC * N), f32)
        for b in range(BPC):
            xT[0:64, b * N:(b + 1) * N] = fe[b][:, None]
            xT[64:67, b * N:(b + 1) * N] = co[b].T
            c4T[0:3, b * N:(b + 1) * N] = co[b].T
        c4T[3, :] = 1.0
        # c4R[jl*4+x, b*8+jc] = coords4[b, jc*16+jl, x]
        co4 = np.concatenate([co, np.ones((BPC, N, 1), f32)], axis=2)
        c4R = co4.reshape(BPC, 8, 16, 4).transpose(2, 3, 0, 1) \
            .reshape(64, BPC * 8)
        in_maps.append({"xT": np.ascontiguousarray(xT).astype(bf16),
                        "c4T": np.ascontiguousarray(c4T).astype(bf16),
                        "c4R": np.ascontiguousarray(c4R),
                        "wb": wb, "wh": wh})
    return in_maps


def kernel(**inputs):
    global _CACHED_NC
    if _CACHED_NC is None:
        _CACHED_NC = build_nc()
    nc = _CACHED_NC
    in_maps = _host_inputs(
        np.asarray(inputs["feature"]), np.asarray(inputs["coordinates_v"]),
        np.asarray(inputs["dw1"]), np.asarray(inputs["db1"]),
        np.asarray(inputs["dw2"]), np.asarray(inputs["db2"]),
        np.asarray(inputs["dw3"]), np.asarray(inputs["db3"]),
        np.asarray(inputs["kw1"]), np.asarray(inputs["kb1"]),
        np.asarray(inputs["kw2"]), np.asarray(inputs["kb2"]),
        np.asarray(inputs["kw3"]), np.asarray(inputs["kb3"]))
    res = run_bass_kernel_spmd(nc, in_maps, list(range(NCORES)))
    out = np.empty((B, N, C2), np.float32)
    for c in range(NCORES):
        # per-core out is [N(i), BPC(b), C2(d)]
        out[c * BPC:(c + 1) * BPC] = res.results[c]["out"].transpose(1, 0, 2)
    return out
